# revision 8
# baseline (speedup 1.0000x reference)
"""AngleAlignmentLoss on 8 TRN2 NeuronCores (Bass, raw-engine SPMD kernel).

Math: for each row i with group g=(target_i, sub_i):
  centers c_g = mean of inputs in group g
  vecs_i[j]  = (x_j - c_g) / max(||x_j - c_g||, eps)
  ang_i[j,k] = vecs_i[j] . vecs_i[k]
  loss = mean_i,m,k | ang_i[idx0[i,m],k] - ang_i[idx1[i,m],k] |

Expansion used on device (all float math on device, index prep on host):
  Gt_i[j,k] = (x_j-c_g).(x_k-c_g) = G[j,k] - P[j,g] - P[k,g] + Qd[g]
  with G = X X^T, P = X C^T, Qd[g] = ||c_g||^2
  ang_i[j,k] = R[g,j] R[g,k] Gt_i[j,k],  R[g,j] = nmask/max(sqrt(relu(N)),eps),
  N[g,j] = G[j,j] - 2 P[j,g] + Qd[g]
  contrib(i) = sum_{m,k} | R[g,k] * ( sum_j W[j,m] Gt_i[j,k] ) |
  with W[j,m] = R[g,j] * (S0 - S1)[j,m]   (host-built +-1 selection diff)
  sum_j W[j,m] Gt[j,k] = (G^T W)[k,m] + (Qd[g]-P[k,g])*s_m - c2_m
  where s_m = sum_j W[j,m], c2_m = sum_j W[j,m] P[j,g].

contrib(i) only depends on g, so we compute one contrib per distinct group
(<=16) weighted by group size: 2 group-slots per core across 8 cores.
"""

import sys

import numpy as np

sys.path.insert(0, "/opt/trn_rl_repo")

import concourse.bass as bass
import concourse.mybir as mybir
from concourse.bass_utils import run_bass_kernel_spmd

f32 = mybir.dt.float32
B = 128
D = 2048
DC = D // 128  # 16 d-chunks
NG = 16        # group slots total
NCORE = 8
SPC = NG // NCORE  # group slots per core = 2
EPS = 1e-12


def _host_prep(inputs, targets, subs, n0):
    x = np.ascontiguousarray(np.asarray(inputs, dtype=np.float32))
    t = np.asarray(targets).astype(np.int64).ravel()
    s = np.asarray(subs).astype(np.int64).ravel()
    n0 = int(np.asarray(n0))
    assert x.shape == (B, D), x.shape
    assert 1 <= n0 <= 64, n0

    pairs = np.stack([t, s], 1)
    uniq, ginv = np.unique(pairs, axis=0, return_inverse=True)
    Gn = uniq.shape[0]
    assert Gn <= NG, f"more than {NG} (target,sub) groups: {Gn}"
    counts = np.bincount(ginv, minlength=NG).astype(np.int64)  # padded to NG

    # M^T: [B, NG], column g = 1/count_g on members of g
    mt = np.zeros((B, NG), np.float32)
    mt[np.arange(B), ginv] = (1.0 / counts[ginv]).astype(np.float32)

    # nmask^T [NG, B]: 0 where j is the sole member of group g (v_j == 0 there)
    nmT = np.ones((NG, B), np.float32)
    for g in range(Gn):
        if counts[g] == 1:
            nmT[g, ginv == g] = 0.0

    # per-group selection-difference sdiff_g [B, n0]
    sd_groups = np.zeros((NG, B, n0), np.float32)
    for g in range(Gn):
        tg = uniq[g, 0]
        mask0 = (t != tg) & (s == 0)
        mask1 = (t != tg) & (s == 1)
        idx0 = np.argsort(~mask0, kind="stable")[:n0]
        idx1 = np.argsort(~mask1, kind="stable")[:n0]
        np.add.at(sd_groups[g], (idx0, np.arange(n0)), 1.0)
        np.add.at(sd_groups[g], (idx1, np.arange(n0)), -1.0)

    i16 = np.eye(NG, dtype=np.float32)
    id128 = np.eye(128, dtype=np.float32)
    xt = np.ascontiguousarray(x.T)

    in_maps = []
    for c in range(NCORE):
        ht = np.zeros((NG, SPC), np.float32)
        wt = np.zeros((SPC, 1), np.float32)
        sd = np.zeros((B, SPC * n0), np.float32)
        for sl in range(SPC):
            g = c * SPC + sl
            if g < Gn and counts[g] > 0:
                ht[g, sl] = 1.0
                wt[sl, 0] = float(counts[g])
                sd[:, sl * n0:(sl + 1) * n0] = sd_groups[g]
        in_maps.append({
            "x": x, "xt": xt, "mt": mt, "nmT": nmT, "i16": i16,
            "id128": id128, "ht": ht, "wt": wt, "sd": sd,
        })
    scale = 1.0 / (B * n0 * B)
    return in_maps, scale


def _build_graph(n0, scale):
    nc = bass.Bass()

    x_ext = nc.declare_dram_parameter("x", [B, D], f32, isOutput=False)
    xt_ext = nc.declare_dram_parameter("xt", [D, B], f32, isOutput=False)
    mt_ext = nc.declare_dram_parameter("mt", [B, NG], f32, isOutput=False)
    nmT_ext = nc.declare_dram_parameter("nmT", [NG, B], f32, isOutput=False)
    i16_ext = nc.declare_dram_parameter("i16", [NG, NG], f32, isOutput=False)
    id128_ext = nc.declare_dram_parameter("id128", [128, 128], f32, isOutput=False)
    ht_ext = nc.declare_dram_parameter("ht", [NG, SPC], f32, isOutput=False)
    wt_ext = nc.declare_dram_parameter("wt", [SPC, 1], f32, isOutput=False)
    sd_ext = nc.declare_dram_parameter("sd", [B, SPC * n0], f32, isOutput=False)
    out_ext = nc.declare_dram_parameter("out", [1, 1], f32, isOutput=True)

    sb = nc.alloc_sbuf_tensor
    xs = sb("xs", [128, D], f32)          # X natural [j, d]
    xts = sb("xts", [128, D], f32)        # X^T tiles: [:, 128c:+128] = XT chunk c
    mts = sb("mts", [B, NG], f32)
    nmTs = sb("nmTs", [NG, B], f32)
    i16s = sb("i16s", [NG, NG], f32)
    id128s = sb("id128s", [128, 128], f32)
    hts = sb("hts", [NG, SPC], f32)
    wts = sb("wts", [SPC, 1], f32)
    sds = sb("sds", [B, SPC * n0], f32)
    Gs = sb("Gs", [128, 128], f32)
    CTs = sb("CTs", [128, DC * NG], f32)  # CT tile c at [:, NG*c:+NG]
    GI = sb("GI", [128, 128], f32)        # scratch for diag extract
    Gd = sb("Gd", [128, 1], f32)
    Gdrow = sb("Gdrow", [1, 128], f32)
    P_sb = sb("P_sb", [128, NG], f32)
    PTs = sb("PTs", [NG, 128], f32)
    QI = sb("QI", [NG, NG], f32)
    Qd = sb("Qd", [NG, 1], f32)
    nt0 = sb("nt0", [NG, 128], f32)
    nt1 = sb("nt1", [NG, 128], f32)
    RT = sb("RT", [NG, 128], f32)
    PT2 = sb("PT2", [NG, 128], f32)
    RLs = sb("RLs", [128, SPC], f32)
    PLs = sb("PLs", [128, SPC], f32)
    PT2Lf = sb("PT2Lf", [1, SPC * 128], f32)
    Wsb = sb("Wsb", [128, SPC * n0], f32)
    ss_sb = sb("ss_sb", [1, SPC * n0], f32)
    cs_sb = sb("cs_sb", [1, SPC * n0], f32)
    Vt = sb("Vt", [128, n0], f32)
    racc = sb("racc", [128, SPC], f32)
    t_sb = sb("t_sb", [SPC, 1], f32)
    outsb = sb("outsb", [1, 1], f32)
    ones_col = sb("ones_col", [128, 1], f32)
    ones16 = sb("ones16", [1, NG], f32)
    ones128 = sb("ones128", [1, 128], f32)

    # One PSUM tensor per accumulation target (sim tracks accumulation
    # groups per tensor, and reads of a tensor with an open group fault).
    ps = nc.alloc_psum_tensor
    PS_G = ps("PS_G", [128, 128], f32)    # G accum; later RL [:,0:SPC], PL [:,32:32+SPC]
    PS_CT = ps("PS_CT", [128, 16], f32)   # CT ping
    PS_CT2 = ps("PS_CT2", [128, 16], f32) # CT pong
    PS_P128 = ps("PS_P128", [128, 16], f32)  # P natural accum
    PS_Q = ps("PS_Q", [16, 512], f32)     # Q [0:16,0:16]; sp/cp rows per slot
    PS_T = ps("PS_T", [128, 512], f32)    # GdT [0:1,0:128]; PT [0:16,128:256]; GdB [0:16,256:384]; PT2L rows; slotTot; tp
    PS_B = ps("PS_B", [128, 64], f32)     # Mi slot even
    PS_B2 = ps("PS_B2", [128, 64], f32)   # Mi slot odd

    INCN = {"dma": 16, "pe": 1, "act": 1, "dve": 1}
    C = {k: 0 for k in INCN}
    prog = []

    def S(eng, emit, waits=(), inc=None):
        if isinstance(waits, dict):
            w = dict(waits)
        else:
            w = {s: C[s] for s in waits if C[s] > 0}
        prog.append((eng, w, emit, inc))
        if inc:
            C[inc] += INCN[inc]
        return dict(C)

    add = mybir.AluOpType.add
    mult = mybir.AluOpType.mult
    AX = mybir.AxisListType.X
    AF = mybir.ActivationFunctionType

    # ---- DMA in (ordered so big/early-needed tensors go first) ----
    def dma(dst, src):
        return lambda e, dst=dst, src=src: e.dma_start(out=dst, in_=src)

    m_xt = S("sync", dma(xts[:, :].rearrange("p (c j) -> p c j", c=DC),
                         xt_ext[:, :].rearrange("(c p) j -> p c j", p=128)),
             inc="dma")["dma"]
    m_x = S("sync", dma(xs[:, :], x_ext[:, :]), inc="dma")["dma"]
    m_mt = S("sync", dma(mts[:, :], mt_ext[:, :]), inc="dma")["dma"]
    m_i16 = S("sync", dma(i16s[:, :], i16_ext[:, :]), inc="dma")["dma"]
    m_id = S("sync", dma(id128s[:, :], id128_ext[:, :]), inc="dma")["dma"]
    m_nm = S("sync", dma(nmTs[:, :], nmT_ext[:, :]), inc="dma")["dma"]
    m_ht = S("sync", dma(hts[:, :], ht_ext[:, :]), inc="dma")["dma"]
    m_wt = S("sync", dma(wts[:, :], wt_ext[:, :]), inc="dma")["dma"]
    m_sd = S("sync", dma(sds[:, :], sd_ext[:, :]), inc="dma")["dma"]
    # HWDGE completion order across DMAs is not guaranteed; with one shared
    # semaphore only the full total (= all input DMAs done) is a safe wait.
    M_IN = C["dma"]
    m_xt = m_x = m_mt = m_i16 = m_id = m_nm = m_ht = m_wt = m_sd = M_IN

    # ---- constants (DVE memsets) ----
    S("dve", lambda e: e.memset(ones_col[:, :], 1.0), inc="dve")
    S("dve", lambda e: e.memset(ones16[:, :], 1.0), inc="dve")
    ones_done = S("dve", lambda e: e.memset(ones128[:, :], 1.0), inc="dve")["dve"]

    # ---- G = X X^T (16 accumulating matmuls over d-chunks) ----
    for c in range(DC):
        S("pe",
          lambda e, c=c: e.matmul(PS_G[:, :], xts[:, 128 * c:128 * (c + 1)],
                                  xts[:, 128 * c:128 * (c + 1)],
                                  start=(c == 0), stop=(c == DC - 1)),
          waits={"dma": m_xt} if c == 0 else (), inc="pe")
    S("act", lambda e: e.copy(Gs[:, :], PS_G[:, :]), waits=("pe",), inc="act")
    act_gs = C["act"]

    # ---- CT tiles: CT_c = X[:,chunk]^T M^T  (one matmul each, ping-pong) ----
    act_ct = [0] * DC
    for c in range(DC):
        slot = PS_CT if c % 2 == 0 else PS_CT2
        w = {"dma": m_mt}
        if c >= 2:
            w = {"dma": m_mt, "act": act_ct[c - 2]}
        S("pe",
          lambda e, c=c, slot=slot: e.matmul(slot[:, 0:NG],
                                             xs[:, 128 * c:128 * (c + 1)],
                                             mts[:, :], start=True, stop=True),
          waits=w, inc="pe")
        act_ct[c] = S("act",
                      lambda e, c=c, slot=slot: e.copy(CTs[:, NG * c:NG * (c + 1)],
                                                       slot[:, 0:NG]),
                      waits=("pe",), inc="act")["act"]

    # ---- P natural [j,g] = sum_c XT_c^T CT_c ----
    for c in range(DC):
        S("pe",
          lambda e, c=c: e.matmul(PS_P128[:, :], xts[:, 128 * c:128 * (c + 1)],
                                  CTs[:, NG * c:NG * (c + 1)],
                                  start=(c == 0), stop=(c == DC - 1)),
          waits={"act": act_ct[c]}, inc="pe")
    S("act", lambda e: e.copy(P_sb[:, :], PS_P128[:, :]), waits=("pe",), inc="act")

    # ---- Q = C C^T  -> [16,16] ----
    for c in range(DC):
        S("pe",
          lambda e, c=c: e.matmul(PS_Q[0:16, 0:16], CTs[:, NG * c:NG * (c + 1)],
                                  CTs[:, NG * c:NG * (c + 1)],
                                  start=(c == 0), stop=(c == DC - 1)),
          inc="pe")
    # Qd = diag(Q) via identity mask + row reduce (DVE, reads PSUM Q)
    S("dve", lambda e: e.tensor_mul(QI[:, :], i16s[:, :], PS_Q[0:16, 0:16]),
      waits=("pe", "dma"), inc="dve")
    S("dve", lambda e: e.tensor_reduce(Qd[:, :], QI[:, :], AX, add), inc="dve")

    # ---- Gd = diag(G) via id mask + reduce (DVE reads PS_G before reuse) ----
    S("dve", lambda e: e.tensor_mul(GI[:, :], id128s[:, :], PS_G[:, :]),
      waits={"dma": m_id, "pe": C["pe"]}, inc="dve")
    S("dve", lambda e: e.tensor_reduce(Gd[:, :], GI[:, :], AX, add), inc="dve")

    # ---- Gd -> row [1,128] via PE transpose, then GdB = ones16 x Gdrow ----
    S("pe", lambda e: e.transpose(PS_T[0:1, 0:128], Gd[:, :], id128s[:, :]),
      waits=("dve",), inc="pe")
    S("act", lambda e: e.copy(Gdrow[:, :], PS_T[0:1, 0:128]), waits=("pe",), inc="act")
    S("pe", lambda e: e.matmul(PS_T[0:16, 256:384], ones16[:, :], Gdrow[:, :],
                               start=True, stop=True),
      waits=("act", "dve"), inc="pe")

    # ---- PT = P^T via PE transpose ----
    S("pe", lambda e: e.transpose(PS_T[0:16, 128:256], P_sb[:, :], id128s[:, :]),
      waits=("act",), inc="pe")
    S("act", lambda e: e.copy(PTs[:, :], PS_T[0:16, 128:256]), waits=("pe",), inc="act")

    # ---- N^T = Gd[j] - 2 PT + Qd[g] ; R = nmask/max(sqrt(relu(N)),eps) ----
    S("dve",
      lambda e: e.tensor_scalar(nt0[:, :], PS_T[0:16, 128:256], -2.0, Qd[:, 0:1],
                                mult, add),
      waits=("pe", "act"), inc="dve")
    S("dve", lambda e: e.tensor_add(nt0[:, :], nt0[:, :], PS_T[0:16, 256:384]),
      inc="dve")
    S("act", lambda e: e.activation(nt1[:, :], nt0[:, :], AF.Relu),
      waits=("dve",), inc="act")
    S("act", lambda e: e.activation(nt1[:, :], nt1[:, :], AF.Sqrt), inc="act")
    S("dve", lambda e: e.tensor_scalar_max(nt1[:, :], nt1[:, :], EPS),
      waits=("act",), inc="dve")
    S("dve", lambda e: e.reciprocal(nt0[:, :], nt1[:, :]), inc="dve")
    S("dve", lambda e: e.tensor_mul(RT[:, :], nt0[:, :], nmTs[:, :]),
      waits={"dma": m_nm}, inc="dve")

    # ---- PT2 = Qd[g] - PT ----
    S("dve", lambda e: e.tensor_scalar(PT2[:, :], PTs[:, :], -1.0, Qd[:, 0:1],
                                       mult, add), inc="dve")

    # ---- per-slot tables: RL, PL [128,SPC]; PT2L [SPC,128] ----
    S("pe", lambda e: e.matmul(PS_G[:, 0:SPC], RT[:, :], hts[:, :],
                               start=True, stop=True),
      waits={"dve": C["dve"], "act": act_gs, "dma": m_ht}, inc="pe")
    S("pe", lambda e: e.matmul(PS_G[:, 32:32 + SPC], PTs[:, :], hts[:, :],
                               start=True, stop=True), inc="pe")
    # PT2L row for slot 0 -> PS_T[0:1,384:512]; slot 1 reuses PS_T[0:1,0:128]
    pt2l_region = [PS_T[0:1, 384:512], PS_T[0:1, 0:128]]
    for sl in range(SPC):
        S("pe",
          lambda e, sl=sl: e.matmul(pt2l_region[sl],
                                    hts[:, sl:sl + 1], PT2[:, :],
                                    start=True, stop=True), inc="pe")
    S("act", lambda e: e.copy(RLs[:, :], PS_G[:, 0:SPC]), waits=("pe",), inc="act")
    S("act", lambda e: e.copy(PLs[:, :], PS_G[:, 32:32 + SPC]), inc="act")
    act_tbl = C["act"]
    for sl in range(SPC):
        act_tbl = S("act",
                    lambda e, sl=sl: e.copy(PT2Lf[0:1, 128 * sl:128 * (sl + 1)],
                                            pt2l_region[sl]),
                    inc="act")["act"]

    # ---- per-slot loop ----
    dve_after_vt = [0, 0]
    for sl in range(SPC):
        psb = PS_B if sl % 2 == 0 else PS_B2
        wcol = slice(sl * n0, (sl + 1) * n0)
        S("dve",
          lambda e, sl=sl, wcol=wcol: e.tensor_scalar_mul(
              Wsb[:, wcol], sds[:, wcol], RLs[:, sl:sl + 1]),
          waits={"act": act_tbl, "dma": m_sd}, inc="dve")
        S("pe",
          lambda e, sl=sl, wcol=wcol: e.matmul(
              PS_Q[0:1, 32 + 256 * sl:32 + 256 * sl + n0],
              ones_col[:, :], Wsb[:, wcol], start=True, stop=True),
          waits=("dve",), inc="pe")
        S("pe",
          lambda e, sl=sl, wcol=wcol: e.matmul(
              PS_Q[0:1, 32 + 256 * sl + 128:32 + 256 * sl + 128 + n0],
              PLs[:, sl:sl + 1], Wsb[:, wcol], start=True, stop=True),
          inc="pe")
        S("act",
          lambda e, sl=sl, wcol=wcol: e.copy(
              ss_sb[:, wcol], PS_Q[0:1, 32 + 256 * sl:32 + 256 * sl + n0]),
          waits=("pe",), inc="act")
        S("act",
          lambda e, sl=sl, wcol=wcol: e.mul(
              cs_sb[:, wcol],
              PS_Q[0:1, 32 + 256 * sl + 128:32 + 256 * sl + 128 + n0], -1.0),
          inc="act")
        S("pe",
          lambda e, sl=sl, wcol=wcol, psb=psb: e.matmul(
              psb[:, 0:n0], Gs[:, :], Wsb[:, wcol], start=True, stop=False),
          inc="pe")
        S("pe",
          lambda e, sl=sl, wcol=wcol, psb=psb: e.matmul(
              psb[:, 0:n0], PT2Lf[0:1, 128 * sl:128 * (sl + 1)], ss_sb[:, wcol],
              start=False, stop=False),
          waits=("act",), inc="pe")
        S("pe",
          lambda e, sl=sl, wcol=wcol, psb=psb: e.matmul(
              psb[:, 0:n0], ones128[:, :], cs_sb[:, wcol],
              start=False, stop=True),
          inc="pe")
        S("dve",
          lambda e, sl=sl, psb=psb: e.tensor_scalar_mul(
              Vt[:, :], psb[:, 0:n0], RLs[:, sl:sl + 1]),
          waits=("pe",), inc="dve")
        dve_after_vt[sl % 2] = S(
            "dve",
            lambda e, sl=sl: e.tensor_reduce(racc[:, sl:sl + 1], Vt[:, :], AX,
                                             add, apply_absolute_value=True),
            inc="dve")["dve"]

    # ---- total = sum_k racc ; weighted by slot counts; scaled ----
    S("pe", lambda e: e.matmul(PS_T[0:SPC, 130:131], racc[:, :], ones_col[:, :],
                               start=True, stop=True),
      waits=("dve",), inc="pe")
    S("act", lambda e: e.copy(t_sb[:, :], PS_T[0:SPC, 130:131]),
      waits=("pe",), inc="act")
    S("pe", lambda e: e.matmul(PS_T[0:1, 140:141], t_sb[:, :], wts[:, :],
                               start=True, stop=True),
      waits={"act": C["act"], "dma": m_wt}, inc="pe")
    S("act", lambda e, scale=scale: e.mul(outsb[:, :], PS_T[0:1, 140:141], scale),
      waits=("pe",), inc="act")
    S("sync", lambda e: e.dma_start(out=out_ext[:, :], in_=outsb[:, :]),
      waits=("act",), inc="dma")

    # ---- emit per-engine streams ----
    with (
        nc.semaphore("dma_sem") as dma_sem,
        nc.semaphore("pe_sem") as pe_sem,
        nc.semaphore("act_sem") as act_sem,
        nc.semaphore("dve_sem") as dve_sem,
        nc.Block() as block,
    ):
        sem_obj = {"dma": dma_sem, "pe": pe_sem, "act": act_sem, "dve": dve_sem}

        def runner(name):
            # dve/act pipelines don't order write->read between their own
            # back-to-back instructions; serialize those streams on their
            # own semaphore.
            self_serialize = name in ("dve", "act")

            def body(eng):
                n_done = 0
                for e, w, emit, inc in prog:
                    if e != name:
                        continue
                    if self_serialize and n_done > 0:
                        eng.wait_ge(sem_obj[name], n_done)
                    for sname, val in w.items():
                        if sname == name:
                            continue
                        if val > 0:
                            eng.wait_ge(sem_obj[sname], val)
                    ins = emit(eng)
                    if inc:
                        ins.then_inc(sem_obj[inc], INCN[inc])
                        n_done += 1
            return body

        block.sync(runner("sync"))
        block.tensor(runner("pe"))
        block.scalar(runner("act"))
        block.vector(runner("dve"))

    return nc


_CACHE = {}


def kernel(**inputs) -> np.ndarray:
    n0 = int(np.asarray(inputs["n0"]))
    in_maps, scale = _host_prep(inputs["inputs"], inputs["targets"],
                                inputs["subs"], n0)
    key = (n0,)
    if key not in _CACHE:
        _CACHE[key] = _build_graph(n0, scale)
    nc = _CACHE[key]
    res = run_bass_kernel_spmd(nc, in_maps, list(range(NCORE)))
    total = np.float32(0.0)
    for c in range(NCORE):
        total += np.float32(res.results[c]["out"].reshape(()))
    return np.float32(total)
